# revision 5
# baseline (speedup 1.0000x reference)
"""AdaptiveSoftmaxProductHead.loss on 8 TRN2 NeuronCores (data-parallel).

Strategy
--------
Per-sample target log-prob = (head target logit - head logsumexp)
                           + [cluster: tail target logit - tail cluster logsumexp].

Host: assigns each of the 2048 samples to one of 8 cores, into one of two
128-slot tiles per core (tile A: cluster-2 + shortlist + cluster-0 overflow;
tile B: cluster-1 + cluster-0).  Gathers the per-sample target weight rows on
the host (pure data movement) so the device never needs data-dependent
indexing.  All device inputs are packed on the host into the exact SBUF
layout so each resident tensor loads with one (or few) large DMAs.

Device (identical SPMD program on every core, different data):
  - head logits for all 256 slots via TensorE (bf16); ScalarE exps them,
    DVE row-sums the exp scratch (tensor_scalar accum) -> softmax denom.
  - tail cluster logits [128 slots, osz] in 512-col PSUM chunks; 4 PE
    row-tiles (small-K packing) fill a 4-bank PSUM group; ScalarE exps a
    [128, 2048] span per instruction; DVE accumulates per-slot row sums
    (keeps the ACT engine free of accumulator-readout overhead).
    Two tail clusters share one 128-slot tile via zero-masked stationaries
    and PSUM accumulate (per-element has_written semantics).
  - target logits via per-slot dot products against host-gathered rows:
    elementwise product (VectorE) then a partition-dim ones-matmul (TensorE).
  - Ln + combine on device; host only unpermutes / adds the two parts.
"""

import numpy as np

# ---------------- problem constants (hardcoded; kernel.py is self-contained) ----
N, D = 2048, 512
SHORT = 1000
CUT = [1000, 10000, 50000, 100000]
OSZ = [9000, 40000, 50000]
HSZ = [128, 32, 8]
NCORES = 8
NSLOT = 128          # slots per tile
GRP = 2048           # columns per ScalarE exp instruction (4 PSUM banks)
P0, P1, P2 = 10240, 40960, 51200   # padded tail column counts
PH = 1024                          # padded head columns
G0, G1, G2 = P0 // GRP, P1 // GRP, P2 // GRP   # 5, 20, 25 ACT groups
Q1, Q2 = P1 // 4, P2 // 4          # per-quarter cols: 10240, 12800
ACC_COLS = 32
# w2_2 quarter chunks (in ACT groups of 512 cols), group-aligned.
# chunk 0 = the G0 cluster-0-overlay groups (processed LAST in ring A);
# chunk 1 is a single group so the ring can start as early as possible.
W22_G = [5, 1, 3, 4, 6, 6]         # per-chunk group counts (sum = 25)

_BUILT = None   # compiled Bass singleton


# ================================ host-side prep ================================

def _assign(targets):
    """Per-core slot lists (tileA, tileB), each length-128 of sample idx or -1."""
    t = targets
    cl = np.where(t < CUT[0], -1,
                  np.where(t < CUT[1], 0, np.where(t < CUT[2], 1, 2)))
    idx_sl = np.nonzero(cl == -1)[0].tolist()
    idx_c = {k: np.nonzero(cl == k)[0].tolist() for k in range(3)}

    tileB = [[] for _ in range(NCORES)]
    for j, i in enumerate(idx_c[1]):
        tileB[j % NCORES].append(i)
    assert all(len(b) <= NSLOT for b in tileB), "cluster-1 per-core overflow"
    c0_ovf = []
    c = 0
    for i in idx_c[0]:
        placed = False
        for d in range(NCORES):
            cc = (c + d) % NCORES
            if len(tileB[cc]) < NSLOT:
                tileB[cc].append(i)
                c = (cc + 1) % NCORES
                placed = True
                break
        if not placed:
            c0_ovf.append(i)

    tileA = [[] for _ in range(NCORES)]
    for j, i in enumerate(idx_c[2]):
        tileA[j % NCORES].append(i)
    assert all(len(a) <= NSLOT for a in tileA), "cluster-2 per-core overflow"
    c = 0
    for i in idx_sl + c0_ovf:
        placed = False
        for d in range(NCORES):
            cc = (c + d) % NCORES
            if len(tileA[cc]) < NSLOT:
                tileA[cc].append(i)
                c = (cc + 1) % NCORES
                placed = True
                break
        assert placed, "tile A overflow"
    for c in range(NCORES):
        tileA[c] += [-1] * (NSLOT - len(tileA[c]))
        tileB[c] += [-1] * (NSLOT - len(tileB[c]))
    return tileA, tileB, cl


def _kpack(mat):
    """[512, C] -> [128, 4*C] with col = k*C + j (K-chunk packing for matmul)."""
    C = mat.shape[1]
    return np.ascontiguousarray(
        mat.reshape(4, 128, C).transpose(1, 0, 2).reshape(128, 4 * C))


def _host_arrays(inputs, tileA, tileB, cl):
    import ml_dtypes
    bf16 = ml_dtypes.bfloat16

    x = np.asarray(inputs["user_repr"], np.float32)
    t = np.asarray(inputs["targets"]).astype(np.int64)
    head_w = np.asarray(inputs["head_w"], np.float32)
    w1 = [np.asarray(inputs[f"tail_w1_{k}"], np.float32) for k in range(3)]
    w2 = [np.asarray(inputs[f"tail_w2_{k}"], np.float32) for k in range(3)]

    def pad_T(w, cols):
        out = np.zeros((w.shape[1], cols), np.float32)
        out[:, : w.shape[0]] = w.T
        return out

    # ---- shared (replicated) weights, packed into SBUF layout ----
    # head: col = (f*4 + k)*512 so each 512-col F-chunk is DMA-contiguous
    hwT = _kpack(pad_T(head_w, PH)).reshape(128, 4, 2, 512)\
        .transpose(0, 2, 1, 3).reshape(128, 4096).copy().astype(bf16)
    w1cat = np.concatenate([w1[0].T, w1[1].T, w1[2].T], axis=1)  # [512, 168]
    w1p = _kpack(w1cat)                                        # [128, 672] f32
    w2_0T = pad_T(w2[0], P0).astype(bf16)                      # [128, 10240]

    def quarters(w, qpad):
        # [osz, hsz] -> [4, hsz, qpad]: osz split into 4 equal quarters,
        # each zero-padded to qpad (so the pad sits at each quarter's END)
        hsz = w.shape[1]
        qr = w.shape[0] // 4
        out = np.zeros((4, hsz, qpad), np.float32)
        out[:, :, :qr] = w.T.reshape(hsz, 4, qr).transpose(1, 0, 2)
        return out

    w2_1p = np.ascontiguousarray(
        quarters(w2[1], Q1).reshape(128, Q1)).astype(bf16)     # [128, 10240]
    # w2_2 in full-128-partition layout: rows 32q + r (r < 8 used, rest zero)
    w2_2p = np.zeros((4, 32, Q2), np.float32)
    w2_2p[:, :8, :] = quarters(w2[2], Q2)
    w2_2p = np.ascontiguousarray(w2_2p.reshape(128, Q2)).astype(bf16)

    in_maps = []
    for c in range(NCORES):
        slots = tileA[c] + tileB[c]
        xs = np.zeros((256, D), np.float32)
        gr = np.zeros((256, D), np.float32)
        for s, i in enumerate(slots):
            if i < 0:
                continue
            xs[s] = x[i]
            ci = cl[i]
            gr[s] = head_w[int(t[i])] if ci == -1 else head_w[SHORT + ci]
        mA2 = np.array([1.0 if (i >= 0 and cl[i] == 2) else 0.0 for i in tileA[c]], np.float32)
        mA0 = np.array([1.0 if (i >= 0 and cl[i] == 0) else 0.0 for i in tileA[c]], np.float32)
        mB1 = np.array([1.0 if (i >= 0 and cl[i] == 1) else 0.0 for i in tileB[c]], np.float32)
        mB0 = np.array([1.0 if (i >= 0 and cl[i] == 0) else 0.0 for i in tileB[c]], np.float32)
        xA, xB = xs[:128], xs[128:]
        # masked transposed x for the four tail stationaries, K-chunk packed:
        # xmask [128, 4m * 4k * 128] with col = ((m*4)+k)*128 + slot
        blocks = []
        for mvec, xt in ((mA2, xA), (mA0, xA), (mB1, xB), (mB0, xB)):
            blocks.append(_kpack(np.ascontiguousarray((xt * mvec[:, None]).T)))
        xmask = np.concatenate(
            [b.reshape(128, 4, 128) for b in blocks], axis=1).reshape(128, 16 * 128)
        # grT: gathered head rows, transposed + K-chunk packed like xT
        grT = _kpack(np.ascontiguousarray(gr.T))                  # [128, 1024]
        # gathered tail target rows, transposed: [hsz, 128] in 4 col-blocks
        g2T = np.zeros((128, 4 * 128), np.float32)
        for s, i in enumerate(tileA[c]):
            if i < 0:
                continue
            if cl[i] == 2:
                g2T[:8, s] = w2[2][int(t[i]) - CUT[2]]
            elif cl[i] == 0:
                g2T[:, 128 + s] = w2[0][int(t[i]) - CUT[0]]
        for s, i in enumerate(tileB[c]):
            if i < 0:
                continue
            if cl[i] == 1:
                g2T[:32, 256 + s] = w2[1][int(t[i]) - CUT[1]]
            elif cl[i] == 0:
                g2T[:, 384 + s] = w2[0][int(t[i]) - CUT[0]]
        # pad corrections: the last ring group is trimmed to the real column
        # count, so cluster-2/1 rows see no zero-pad exps; cluster-0 rows
        # still see P0 - osz0 of them
        corrA = -(P0 - OSZ[0]) * mA0 + (1.0 - mA2 - mA0)
        corrB = -(P0 - OSZ[0]) * mB0 + (1.0 - mB1 - mB0)
        mcpack = np.stack([mA2, mA0, mB1, mB0, corrA, corrB], axis=1)\
            .astype(np.float32)                                   # [128, 6]

        # one packed small-input tensor, ordered so the first DMA split
        # carries exactly what the first projections need:
        # w1p | xmA2 | xmB1 | xmA0 | xmB0 | xT | grT | g2T | ones
        xmr = xmask.reshape(128, 4, 512)
        small = np.concatenate([
            w1p,                                     # [128, 672]
            xmr[:, 0].reshape(128, 512),             # A2
            xmr[:, 2].reshape(128, 512),             # B1
            xmr[:, 1].reshape(128, 512),             # A0
            xmr[:, 3].reshape(128, 512),             # B0
            _kpack(np.ascontiguousarray(xs.T)),      # [128, 1024]
            grT,                                     # [128, 1024]
            g2T,                                     # [128, 512]
            np.ones((128, 16), np.float32),          # [128, 16]
        ], axis=1)
        m = {
            "small": small.astype(bf16),
            "mcpack": mcpack,
            "hwT": hwT,
            "w2_0T": w2_0T, "w2_1p": w2_1p, "w2_2p": w2_2p,
        }
        in_maps.append(m)
    return in_maps


# ================================ device program ================================

def build_nc():
    from concourse import bacc, tile
    import concourse.mybir as mybir

    bf = mybir.dt.bfloat16
    f32 = mybir.dt.float32
    AF = mybir.ActivationFunctionType
    ALU = mybir.AluOpType
    AX = mybir.AxisListType

    nc = bacc.Bacc("TRN2", target_bir_lowering=False, debug=False,
                   num_devices=NCORES)

    dp = nc.declare_dram_parameter
    SMALL_C = 2048 + 672 + 1024 + 1024 + 512 + 16
    d_small = dp("small", [128, SMALL_C], bf, False)
    d_mc = dp("mcpack", [128, 6], f32, False)
    d_hwT = dp("hwT", [128, 4 * PH], bf, False)
    d_w2_0T = dp("w2_0T", [HSZ[0], P0], bf, False)
    d_w2_1p = dp("w2_1p", [128, Q1], bf, False)
    d_w2_2p = dp("w2_2p", [128, Q2], bf, False)
    d_out = dp("out", [128, 4], f32, True)

    with tile.TileContext(nc) as tc:
        with tc.tile_pool(name="consts", bufs=1) as cp, \
             tc.tile_pool(name="acc", bufs=1) as ap_, \
             tc.tile_pool(name="scr", bufs=2) as sp:

            # ------------- DMA loads --------------------------------------
            # The critical first chunk of w22 is dispatched from the Scalar
            # queue (HWDGE works on SP and Activation) so it goes out in
            # parallel with SP's first dispatch.
            s_w22 = [None] * 6

            def w22_tile(ch, eng):
                cw = W22_G[ch] * 512
                off = sum(W22_G[:ch]) * 512
                t_ = cp.tile([128, cw], bf, name=f"s_w22_{ch}", tag=f"s_w22_{ch}")
                eng.dma_start(out=t_[:, :], in_=d_w2_2p.ap()[:, off:off + cw])
                return t_

            s_w22[1] = w22_tile(1, nc.scalar)

            # pin the table set that holds BOTH Exp and Ln so the tail Ln
            # does not pay a ~2.7us table switch
            nc.scalar.add_instruction(mybir.InstLoadActFuncSet(
                name=nc.get_next_instruction_name(), ins=[], outs=[],
                act_func_set_id=6))

            # SP dispatch order == consumption order.
            s_small = cp.tile([128, SMALL_C], bf, name="s_small", tag="s_small")
            nc.sync.dma_start(out=s_small[:, 0:1184], in_=d_small.ap()[:, 0:1184])
            nc.sync.dma_start(out=s_small[:, 1184:2720],
                              in_=d_small.ap()[:, 1184:2720])
            s_w22[2] = w22_tile(2, nc.sync)
            s_hwT = cp.tile([128, 4096], bf, name="s_hwT", tag="s_hwT")
            nc.sync.dma_start(out=s_hwT[:, :], in_=d_hwT.ap())
            nc.sync.dma_start(out=s_small[:, 2720:SMALL_C],
                              in_=d_small.ap()[:, 2720:SMALL_C])
            s_w22[3] = w22_tile(3, nc.sync)
            s_w22[4] = w22_tile(4, nc.sync)
            s_w22[5] = w22_tile(5, nc.sync)
            s_w20 = cp.tile([128, P0], bf, name="s_w20", tag="s_w20")
            nc.sync.dma_start(out=s_w20[:, :], in_=d_w2_0T.ap())
            s_w22[0] = w22_tile(0, nc.sync)
            s_w21 = cp.tile([128, Q1], bf, name="s_w21", tag="s_w21")
            nc.sync.dma_start(out=s_w21[:, :], in_=d_w2_1p.ap())
            s_mc = cp.tile([128, 6], f32, name="s_mc", tag="s_mc")
            nc.sync.dma_start(out=s_mc[:, :], in_=d_mc.ap())

            s_w1p = s_small[:, 0:672]
            s_xT = s_small[:, 2720:3744]
            s_grT = s_small[:, 3744:4768]
            s_g2T = s_small[:, 4768:5280]
            s_ones = s_small[:, 5280:5281]

            # ------------- views ----------------------------------------------
            def xm(m, k):        # masked-x chunk [128, 128]: m in A2,A0,B1,B0
                base = {0: 672, 1: 1696, 2: 1184, 3: 2208}[m]
                return s_small[:, base + k * 128: base + k * 128 + 128]

            def xTk(k, sl):      # xT chunk [128, 128] for slot range sl
                return s_xT[:, k * 256 + sl * 128: k * 256 + sl * 128 + 128]

            def hwk(k, f):       # head weight chunk [128, 512]
                o = (f * 4 + k) * 512
                return s_hwT[:, o: o + 512]

            def w1k(seg, k):     # w1 seg (0:128 | 1:32 | 2:8) chunk
                off = [0, 128, 160][seg]
                w = HSZ[seg]
                return s_w1p[:, k * 168 + off: k * 168 + off + w]

            mA2_v, mA0_v = s_mc[:, 0:1], s_mc[:, 1:2]
            mB1_v, mB0_v = s_mc[:, 2:3], s_mc[:, 3:4]
            corrA_v, corrB_v = s_mc[:, 4:5], s_mc[:, 5:6]

            # accumulators / combine tiles
            accA = ap_.tile([128, ACC_COLS], f32, name="accA", tag="accA")
            accB = ap_.tile([128, ACC_COLS], f32, name="accB", tag="accB")
            accH = ap_.tile([128, 4], f32, name="accH", tag="accH")
            tgt4 = ap_.tile([128, 4], f32, name="tgt4", tag="tgt4")
            S4 = ap_.tile([128, 4], f32, name="S4", tag="S4")
            ln4 = ap_.tile([128, 4], f32, name="ln4", tag="ln4")
            out4 = ap_.tile([128, 4], f32, name="out4", tag="out4")
            tmp = [ap_.tile([128, 1], f32, name=f"tmp{i}", tag=f"tmp{i}")
                   for i in range(6)]
            junk = ap_.tile([128, ACC_COLS], f32, name="junk", tag="junk")
            s_h2a = ap_.tile([128, 128], bf, name="s_h2a", tag="s_h2a")
            s_h1b = ap_.tile([128, 128], bf, name="s_h1b", tag="s_h1b")
            s_h0a = ap_.tile([128, 128], bf, name="s_h0a", tag="s_h0a")
            s_h0b = ap_.tile([128, 128], bf, name="s_h0b", tag="s_h0b")

            w22_bounds = np.cumsum([0] + W22_G).tolist()

            with tc.tile_pool(name="psR", bufs=2, space="PSUM") as pr:
                # ---- early projections live in one rotating ring slot so
                # they overlap the first ring groups without extra PSUM.
                # Phase 1: only h2a (gates ring A); phase 2 after ring start.
                pe_t = pr.tile([128, GRP], f32, name="proj_ps", tag="ring")
                h2_ps = pe_t[:, 0:128]
                h1_ps = pe_t[:, 128:256]
                h0a_ps = pe_t[:, 256:384]
                h0b_ps = pe_t[:, 384:512]
                for g in range(4):
                    for k in range(4):
                        nc.tensor.matmul(
                            h2_ps[32 * g:32 * g + 8, :], w1k(2, k), xm(0, k),
                            start=(k == 0), stop=(k == 3),
                            tile_position=(0, 32 * g))
                for g in range(4):
                    nc.vector.tensor_copy(s_h2a[32 * g:32 * g + 8, :],
                                          h2_ps[32 * g:32 * g + 8, :])

                # ------------- tail rings: exp + row-sum over cluster logits ---
                # ring A: cluster-2 quarters + cluster-0 overflow (groups < G0).
                # Groups >= G0 (no w2_0 dependency) run first so the ring can
                # start as soon as the first w2_2 chunk lands.

                def exp_and_sum(pt, fw, acc_col):
                    # ScalarE exp (no accumulator readout), then a DVE
                    # tensor_scalar (4x bf16) folds the row sum into acc_col.
                    rsc = sp.tile([128, GRP], bf, name="rscr", tag="rscr")
                    rsm = sp.tile([128, GRP], bf, name="rsum", tag="rsum")
                    if fw == 512:
                        src, dst, red = pt[:, :], rsc[:, :], rsc[:, :]
                        rout = rsm[:, :]
                    else:
                        src = pt.rearrange("p (b e) -> p b e", b=4)[:, :, 0:fw]
                        dst = rsc.rearrange("p (b e) -> p b e", b=4)[:, :, 0:fw]
                        red = dst
                        rout = rsm.rearrange("p (b e) -> p b e", b=4)[:, :, 0:fw]
                    nc.scalar.activation(dst, src, AF.Exp)
                    nc.vector.tensor_scalar(
                        out=rout, in0=red, scalar1=1.0, scalar2=None,
                        op0=ALU.mult, op1=ALU.add, accum_out=acc_col)

                def ringA_group(t):
                    ch = next(i for i in range(len(W22_G))
                              if w22_bounds[i + 1] > t)
                    w = t - w22_bounds[ch]
                    # last group: only 12500 % 512 = 212 real cols per quarter
                    fw = 212 if t == G2 - 1 else 512
                    pt = pr.tile([128, GRP], f32, name="ringA", tag="ring")
                    for g in range(4):
                        nc.tensor.matmul(
                            pt[:, g * 512:g * 512 + fw],
                            s_h2a[32 * g:32 * g + 8, :],
                            s_w22[ch][32 * g:32 * g + 8, w * 512:w * 512 + fw],
                            start=True, stop=(t >= G0),
                            tile_position=(32 * g, 0))
                    if t < G0:
                        for g in range(4):
                            nc.tensor.matmul(
                                pt[:, g * 512:(g + 1) * 512],
                                s_h0a[:, :],
                                s_w20[:, t * GRP + g * 512:t * GRP + (g + 1) * 512],
                                start=False, stop=True)
                    exp_and_sum(pt, fw, accA[:, t:t + 1])

                def head_tile(s):
                    # head logits + exp-sum for sample tile s; one ring slot,
                    # PE cost hides under ring ACT slack
                    hp = pr.tile([128, PH], f32, name=f"head_ps{s}", tag="ring")
                    for f in range(PH // 512):
                        for k in range(4):
                            nc.tensor.matmul(
                                hp[:, f * 512:(f + 1) * 512],
                                xTk(k, s), hwk(k, f),
                                start=(k == 0), stop=(k == 3))
                    hsc = sp.tile([128, PH], bf, name="hscr", tag="rscr")
                    hsm = sp.tile([128, PH], bf, name="hsum", tag="rsum")
                    nc.scalar.activation(hsc[:, :], hp[:, :], AF.Exp)
                    nc.vector.tensor_scalar(
                        out=hsm[:, :], in0=hsc[:, :], scalar1=1.0, scalar2=None,
                        op0=ALU.mult, op1=ALU.add, accum_out=accH[:, s:s + 1])

                # ring A starts on the first w22 chunk; remaining projections
                # (phase 2) run on the PE under the first groups' exp time.
                ringA_group(5)
                for g in range(4):
                    for k in range(4):
                        nc.tensor.matmul(
                            h1_ps[32 * g:32 * g + 32, :], w1k(1, k), xm(2, k),
                            start=(k == 0), stop=(k == 3),
                            tile_position=(0, 32 * g))
                for k in range(4):
                    nc.tensor.matmul(h0a_ps[:, :], w1k(0, k), xm(1, k),
                                     start=(k == 0), stop=(k == 3))
                for k in range(4):
                    nc.tensor.matmul(h0b_ps[:, :], w1k(0, k), xm(3, k),
                                     start=(k == 0), stop=(k == 3))
                for g in range(4):
                    nc.vector.tensor_copy(s_h1b[32 * g:32 * g + 32, :],
                                          h1_ps[32 * g:32 * g + 32, :])
                nc.vector.tensor_copy(s_h0a[:, :], h0a_ps[:, :])
                nc.vector.tensor_copy(s_h0b[:, :], h0b_ps[:, :])

                for t in range(6, G2):
                    ringA_group(t)
                    if t == 9:
                        head_tile(0)
                    elif t == 11:
                        head_tile(1)

                # p-products for the target dots (DVE is idle here; the dot
                # matmuls themselves run after ring B)
                ph = sp.tile([128, 1024], bf, name="ph", tag="ph")
                nc.vector.tensor_mul(ph[:, :], s_xT[:, :], s_grT[:, :])
                p2 = sp.tile([128, 128], bf, name="p2", tag="p2")
                nc.vector.tensor_mul(p2[0:8, :], s_h2a[0:8, :], s_g2T[0:8, 0:128])
                p0a = sp.tile([128, 128], bf, name="p0a", tag="p0a")
                nc.vector.tensor_mul(p0a[:, :], s_h0a[:, :], s_g2T[:, 128:256])
                p1 = sp.tile([128, 128], bf, name="p1", tag="p1")
                nc.vector.tensor_mul(p1[0:32, :], s_h1b[0:32, :], s_g2T[0:32, 256:384])
                p0b = sp.tile([128, 128], bf, name="p0b", tag="p0b")
                nc.vector.tensor_mul(p0b[:, :], s_h0b[:, :], s_g2T[:, 384:512])

                def ringB_group(t):
                    # last group: only 10000 % 512 = 272 real cols per quarter
                    fw = 272 if t == G1 - 1 else 512
                    pt = pr.tile([128, GRP], f32, name="ringB", tag="ring")
                    for g in range(4):
                        nc.tensor.matmul(
                            pt[:, g * 512:g * 512 + fw],
                            s_h1b[32 * g:32 * g + 32, :],
                            s_w21[32 * g:32 * g + 32, t * 512:t * 512 + fw],
                            start=True, stop=(t >= G0),
                            tile_position=(32 * g, 0))
                    if t < G0:
                        for g in range(4):
                            nc.tensor.matmul(
                                pt[:, g * 512:(g + 1) * 512],
                                s_h0b[:, :],
                                s_w20[:, t * GRP + g * 512:t * GRP + (g + 1) * 512],
                                start=False, stop=True)
                    exp_and_sum(pt, fw, accB[:, t:t + 1])

                for t in range(G0):
                    ringA_group(t)

                # ring-A side of the combine (DVE; runs while ring B exps):
                # S_A = mA2*sum(accA[:, :G2]) + mA0*sum(accA[:, :G0]) + corrA,
                # fused as two masked tensor_scalar reduces + one 2-op add.
                nc.vector.tensor_scalar(
                    out=junk[:, 0:G2], in0=accA[:, 0:G2], scalar1=mA2_v,
                    scalar2=None, op0=ALU.mult, op1=ALU.add, accum_out=tmp[0][:, :])
                nc.vector.tensor_scalar(
                    out=junk[:, 0:G0], in0=accA[:, 0:G0], scalar1=mA0_v,
                    scalar2=None, op0=ALU.mult, op1=ALU.add, accum_out=tmp[1][:, :])
                nc.vector.tensor_scalar(
                    out=S4[:, 2:3], in0=tmp[0][:, :], scalar1=tmp[1][:, :],
                    scalar2=corrA_v, op0=ALU.add, op1=ALU.add)
                # head: S = accH - (PH - 1003) pad-exp correction
                nc.vector.tensor_scalar(
                    out=S4[:, 0:2], in0=accH[:, 0:2], scalar1=float(-(PH - 1003)),
                    scalar2=None, op0=ALU.add)

                for t in range(7):
                    ringB_group(t)

                # target-logit dots: per-slot dot(u, v) = (u .* v)^T @ ones
                # (partition-dim contraction on the PE -> [slots, 1] PSUM);
                # one slot in ring B's light region, hidden under its ACT slack
                dots_ps = pr.tile([128, GRP], f32, name="dots_ps", tag="ring")
                for k in range(4):
                    nc.tensor.matmul(dots_ps[:, 0:1], ph[:, k * 256:k * 256 + 128],
                                     s_ones[:, :], start=(k == 0), stop=(k == 3))
                for k in range(4):
                    nc.tensor.matmul(dots_ps[:, 1:2],
                                     ph[:, k * 256 + 128:k * 256 + 256],
                                     s_ones[:, :], start=(k == 0), stop=(k == 3))
                nc.tensor.matmul(dots_ps[:, 2:3], p2[0:8, :], s_ones[0:8, :],
                                 start=True, stop=False)
                nc.tensor.matmul(dots_ps[:, 2:3], p0a[:, :], s_ones[:, :],
                                 start=False, stop=True)
                nc.tensor.matmul(dots_ps[:, 3:4], p1[0:32, :], s_ones[0:32, :],
                                 start=True, stop=False)
                nc.tensor.matmul(dots_ps[:, 3:4], p0b[:, :], s_ones[:, :],
                                 start=False, stop=True)
                nc.vector.tensor_copy(tgt4[:, :], dots_ps[:, 0:4])

                for t in range(7, G1):
                    ringB_group(t)

            # ------------- combine (only ring-B accB remains) ----------------
            nc.vector.tensor_scalar(
                out=junk[:, 0:G1], in0=accB[:, 0:G1], scalar1=mB1_v,
                scalar2=None, op0=ALU.mult, op1=ALU.add, accum_out=tmp[2][:, :])
            nc.vector.tensor_scalar(
                out=junk[:, 0:G0], in0=accB[:, 0:G0], scalar1=mB0_v,
                scalar2=None, op0=ALU.mult, op1=ALU.add, accum_out=tmp[3][:, :])
            nc.vector.tensor_scalar(
                out=S4[:, 3:4], in0=tmp[2][:, :], scalar1=tmp[3][:, :],
                scalar2=corrB_v, op0=ALU.add, op1=ALU.add)
            nc.scalar.activation(ln4[:, :], S4[:, :], AF.Ln)
            nc.vector.tensor_sub(out4[:, :], tgt4[:, :], ln4[:, :])
            nc.sync.dma_start(out=d_out.ap(), in_=out4[:, :])

    nc.compile()
    return nc


def _get_nc():
    global _BUILT
    if _BUILT is None:
        _BUILT = build_nc()
    return _BUILT


# ================================ entry point ================================

def _numpy_fallback(inputs):
    """Last-resort exact computation (only if the slot assignment misfits,
    which cannot happen for the deterministic problem inputs)."""
    x = np.asarray(inputs["user_repr"], np.float64)
    t = np.asarray(inputs["targets"]).astype(np.int64)
    head_w = np.asarray(inputs["head_w"], np.float64)
    rows = np.arange(x.shape[0])

    def lse_rows(logits):
        m = logits.max(axis=1, keepdims=True)
        return (np.log(np.exp(logits - m).sum(axis=1, keepdims=True)) + m)

    hl = x @ head_w.T
    head_lp = hl - lse_rows(hl)
    out = np.where(t < SHORT, head_lp[rows, np.minimum(t, SHORT - 1)], 0.0)
    for i in range(3):
        w1 = np.asarray(inputs[f"tail_w1_{i}"], np.float64)
        w2 = np.asarray(inputs[f"tail_w2_{i}"], np.float64)
        tl = (x @ w1.T) @ w2.T
        tail_lp = tl - lse_rows(tl)
        rel = np.clip(t - CUT[i], 0, CUT[i + 1] - CUT[i] - 1)
        val = head_lp[:, SHORT + i] + tail_lp[rows, rel]
        out = np.where((t >= CUT[i]) & (t < CUT[i + 1]), val, out)
    return out.astype(np.float32)


def kernel(**inputs):
    from concourse.bass_utils import run_bass_kernel_spmd

    targets = np.asarray(inputs["targets"]).astype(np.int64)
    try:
        tileA, tileB, cl = _assign(targets)
    except AssertionError:
        return _numpy_fallback(inputs)
    in_maps = _host_arrays(inputs, tileA, tileB, cl)
    nc = _get_nc()
    res = run_bass_kernel_spmd(nc, in_maps, core_ids=list(range(NCORES)))
    out = np.zeros(N, np.float32)
    for c in range(NCORES):
        o = res.results[c]["out"]   # [128, 4]
        for s, i in enumerate(tileA[c]):
            if i >= 0:
                out[i] = o[s, 0] + (o[s, 2] if cl[i] >= 0 else 0.0)
        for s, i in enumerate(tileB[c]):
            if i >= 0:
                out[i] = o[s, 1] + o[s, 3]
    return out


# revision 7
# speedup vs baseline: 1.0867x; 1.0867x over previous
"""AdaptiveSoftmaxProductHead.loss on 8 TRN2 NeuronCores (data-parallel).

Strategy
--------
Per-sample target log-prob = (head target logit - head logsumexp)
                           + [cluster: tail target logit - tail cluster logsumexp].

Host: assigns each of the 2048 samples to one of 8 cores, into one of two
128-slot tiles per core (tile A: cluster-2 + shortlist + cluster-0 overflow;
tile B: cluster-1 + cluster-0).  Gathers the per-sample target weight rows on
the host (pure data movement) so the device never needs data-dependent
indexing.  All device inputs are packed on the host into the exact SBUF
layout so each resident tensor loads with one (or few) large DMAs.

Device (identical SPMD program on every core, different data):
  - head logits for all 256 slots via TensorE (bf16); ScalarE exps them,
    DVE row-sums the exp scratch (tensor_scalar accum) -> softmax denom.
  - tail cluster logits [128 slots, osz] in 512-col PSUM chunks; 4 PE
    row-tiles (small-K packing) fill a 4-bank PSUM group; ScalarE exps a
    [128, 2048] span per instruction; DVE accumulates per-slot row sums
    (keeps the ACT engine free of accumulator-readout overhead).
    Two tail clusters share one 128-slot tile via zero-masked stationaries
    and PSUM accumulate (per-element has_written semantics).
  - target logits via per-slot dot products against host-gathered rows:
    elementwise product (VectorE) then a partition-dim ones-matmul (TensorE).
  - Ln + combine on device; host only unpermutes / adds the two parts.
"""

import numpy as np

# ---------------- problem constants (hardcoded; kernel.py is self-contained) ----
N, D = 2048, 512
SHORT = 1000
CUT = [1000, 10000, 50000, 100000]
OSZ = [9000, 40000, 50000]
HSZ = [128, 32, 8]
NCORES = 8
NSLOT = 128          # slots per tile
GRP = 2048           # columns per ScalarE exp instruction (4 PSUM banks)
P0, P1, P2 = 10240, 40960, 51200   # padded tail column counts
PH = 1024                          # padded head columns
G0, G1, G2 = P0 // GRP, P1 // GRP, P2 // GRP   # 5, 20, 25 ACT groups
Q1, Q2 = P1 // 4, P2 // 4          # per-quarter cols: 10240, 12800
ACC_COLS = 32
# w2_2 quarter chunks (in ACT groups of 512 cols), group-aligned.
# chunk 0 = the G0 cluster-0-overlay groups (processed LAST in ring A);
# chunk 1 is a single group so the ring can start as early as possible.
W22_G = [5, 1, 3, 4, 6, 6]         # per-chunk group counts (sum = 25)

_BUILT = None   # compiled Bass singleton


# ================================ host-side prep ================================

def _assign(targets):
    """Per-core slot lists (tileA, tileB), each length-128 of sample idx or -1."""
    t = targets
    cl = np.where(t < CUT[0], -1,
                  np.where(t < CUT[1], 0, np.where(t < CUT[2], 1, 2)))
    idx_sl = np.nonzero(cl == -1)[0].tolist()
    idx_c = {k: np.nonzero(cl == k)[0].tolist() for k in range(3)}

    tileB = [[] for _ in range(NCORES)]
    for j, i in enumerate(idx_c[1]):
        tileB[j % NCORES].append(i)
    assert all(len(b) <= NSLOT for b in tileB), "cluster-1 per-core overflow"
    c0_ovf = []
    c = 0
    for i in idx_c[0]:
        placed = False
        for d in range(NCORES):
            cc = (c + d) % NCORES
            if len(tileB[cc]) < NSLOT:
                tileB[cc].append(i)
                c = (cc + 1) % NCORES
                placed = True
                break
        if not placed:
            c0_ovf.append(i)

    tileA = [[] for _ in range(NCORES)]
    for j, i in enumerate(idx_c[2]):
        tileA[j % NCORES].append(i)
    assert all(len(a) <= NSLOT for a in tileA), "cluster-2 per-core overflow"
    c = 0
    for i in idx_sl + c0_ovf:
        placed = False
        for d in range(NCORES):
            cc = (c + d) % NCORES
            if len(tileA[cc]) < NSLOT:
                tileA[cc].append(i)
                c = (cc + 1) % NCORES
                placed = True
                break
        assert placed, "tile A overflow"
    for c in range(NCORES):
        tileA[c] += [-1] * (NSLOT - len(tileA[c]))
        tileB[c] += [-1] * (NSLOT - len(tileB[c]))
    return tileA, tileB, cl


def _kpack(mat):
    """[512, C] -> [128, 4*C] with col = k*C + j (K-chunk packing for matmul)."""
    C = mat.shape[1]
    return np.ascontiguousarray(
        mat.reshape(4, 128, C).transpose(1, 0, 2).reshape(128, 4 * C))


def _host_arrays(inputs, tileA, tileB, cl):
    import ml_dtypes
    bf16 = ml_dtypes.bfloat16

    x = np.asarray(inputs["user_repr"], np.float32)
    t = np.asarray(inputs["targets"]).astype(np.int64)
    head_w = np.asarray(inputs["head_w"], np.float32)
    w1 = [np.asarray(inputs[f"tail_w1_{k}"], np.float32) for k in range(3)]
    w2 = [np.asarray(inputs[f"tail_w2_{k}"], np.float32) for k in range(3)]

    def pad_T(w, cols):
        out = np.zeros((w.shape[1], cols), np.float32)
        out[:, : w.shape[0]] = w.T
        return out

    # ---- shared (replicated) weights, packed into SBUF layout ----
    # head: col = (f*4 + k)*512 so each 512-col F-chunk is DMA-contiguous
    hwT = _kpack(pad_T(head_w, PH)).reshape(128, 4, 2, 512)\
        .transpose(0, 2, 1, 3).reshape(128, 4096).copy().astype(bf16)
    w1cat = np.concatenate([w1[0].T, w1[1].T, w1[2].T], axis=1)  # [512, 168]
    w1p = _kpack(w1cat)                                        # [128, 672] f32
    w2_0T = pad_T(w2[0], P0).astype(bf16)                      # [128, 10240]

    def quarters(w, qpad):
        # [osz, hsz] -> [4, hsz, qpad]: osz split into 4 equal quarters,
        # each zero-padded to qpad (so the pad sits at each quarter's END)
        hsz = w.shape[1]
        qr = w.shape[0] // 4
        out = np.zeros((4, hsz, qpad), np.float32)
        out[:, :, :qr] = w.T.reshape(hsz, 4, qr).transpose(1, 0, 2)
        return out

    w2_1p = np.ascontiguousarray(
        quarters(w2[1], Q1).reshape(128, Q1)).astype(bf16)     # [128, 10240]
    # w2_2 in full-128-partition layout: rows 32q + r (r < 8 used, rest zero)
    w2_2p = np.zeros((4, 32, Q2), np.float32)
    w2_2p[:, :8, :] = quarters(w2[2], Q2)
    w2_2p = np.ascontiguousarray(w2_2p.reshape(128, Q2)).astype(bf16)

    in_maps = []
    for c in range(NCORES):
        slots = tileA[c] + tileB[c]
        xs = np.zeros((256, D), np.float32)
        gr = np.zeros((256, D), np.float32)
        for s, i in enumerate(slots):
            if i < 0:
                continue
            xs[s] = x[i]
            ci = cl[i]
            gr[s] = head_w[int(t[i])] if ci == -1 else head_w[SHORT + ci]
        mA2 = np.array([1.0 if (i >= 0 and cl[i] == 2) else 0.0 for i in tileA[c]], np.float32)
        mA0 = np.array([1.0 if (i >= 0 and cl[i] == 0) else 0.0 for i in tileA[c]], np.float32)
        mB1 = np.array([1.0 if (i >= 0 and cl[i] == 1) else 0.0 for i in tileB[c]], np.float32)
        mB0 = np.array([1.0 if (i >= 0 and cl[i] == 0) else 0.0 for i in tileB[c]], np.float32)
        xA, xB = xs[:128], xs[128:]
        # masked transposed x for the four tail stationaries, K-chunk packed:
        # xmask [128, 4m * 4k * 128] with col = ((m*4)+k)*128 + slot
        blocks = []
        for mvec, xt in ((mA2, xA), (mA0, xA), (mB1, xB), (mB0, xB)):
            blocks.append(_kpack(np.ascontiguousarray((xt * mvec[:, None]).T)))
        xmask = np.concatenate(
            [b.reshape(128, 4, 128) for b in blocks], axis=1).reshape(128, 16 * 128)
        # grT: gathered head rows, transposed + K-chunk packed like xT
        grT = _kpack(np.ascontiguousarray(gr.T))                  # [128, 1024]
        # gathered tail target rows, transposed: [hsz, 128] in 4 col-blocks
        g2T = np.zeros((128, 4 * 128), np.float32)
        for s, i in enumerate(tileA[c]):
            if i < 0:
                continue
            if cl[i] == 2:
                g2T[:8, s] = w2[2][int(t[i]) - CUT[2]]
            elif cl[i] == 0:
                g2T[:, 128 + s] = w2[0][int(t[i]) - CUT[0]]
        for s, i in enumerate(tileB[c]):
            if i < 0:
                continue
            if cl[i] == 1:
                g2T[:32, 256 + s] = w2[1][int(t[i]) - CUT[1]]
            elif cl[i] == 0:
                g2T[:, 384 + s] = w2[0][int(t[i]) - CUT[0]]
        # pad corrections: the last ring group is trimmed to the real column
        # count, so cluster-2/1 rows see no zero-pad exps; cluster-0 rows
        # still see P0 - osz0 of them
        corrA = -(P0 - OSZ[0]) * mA0 + (1.0 - mA2 - mA0)
        corrB = -(P0 - OSZ[0]) * mB0 + (1.0 - mB1 - mB0)
        mcpack = np.stack([mA2, mA0, mB1, mB0, corrA, corrB], axis=1)\
            .astype(np.float32)                                   # [128, 6]

        # one packed small-input tensor, ordered so the first DMA split
        # carries exactly what the first projections need:
        # w1p | xmA2 | xmB1 | xmA0 | xmB0 | xT | grT | g2T | ones
        xmr = xmask.reshape(128, 4, 512)
        small = np.concatenate([
            w1p,                                     # [128, 672]
            xmr[:, 0].reshape(128, 512),             # A2
            xmr[:, 2].reshape(128, 512),             # B1
            xmr[:, 1].reshape(128, 512),             # A0
            xmr[:, 3].reshape(128, 512),             # B0
            _kpack(np.ascontiguousarray(xs.T)),      # [128, 1024]
            grT,                                     # [128, 1024]
            g2T,                                     # [128, 512]
            np.ones((128, 16), np.float32),          # [128, 16]
        ], axis=1)
        m = {
            "small": small.astype(bf16),
            "mcpack": mcpack,
            "hwT": hwT,
            "w2_0T": w2_0T, "w2_1p": w2_1p, "w2_2p": w2_2p,
        }
        in_maps.append(m)
    return in_maps


# ================================ device program ================================

def build_nc():
    from concourse import bacc, tile
    import concourse.mybir as mybir

    bf = mybir.dt.bfloat16
    f32 = mybir.dt.float32
    AF = mybir.ActivationFunctionType
    ALU = mybir.AluOpType
    AX = mybir.AxisListType

    nc = bacc.Bacc("TRN2", target_bir_lowering=False, debug=False,
                   num_devices=NCORES)

    dp = nc.declare_dram_parameter
    SMALL_C = 2048 + 672 + 1024 + 1024 + 512 + 16
    d_small = dp("small", [128, SMALL_C], bf, False)
    d_mc = dp("mcpack", [128, 6], f32, False)
    d_hwT = dp("hwT", [128, 4 * PH], bf, False)
    d_w2_0T = dp("w2_0T", [HSZ[0], P0], bf, False)
    d_w2_1p = dp("w2_1p", [128, Q1], bf, False)
    d_w2_2p = dp("w2_2p", [128, Q2], bf, False)
    d_out = dp("out", [128, 4], f32, True)

    with tile.TileContext(nc) as tc:
        with tc.tile_pool(name="consts", bufs=1) as cp, \
             tc.tile_pool(name="acc", bufs=1) as ap_, \
             tc.tile_pool(name="scr", bufs=2) as sp:

            # ------------- DMA loads --------------------------------------
            # The critical first chunk of w22 is dispatched from the Scalar
            # queue (HWDGE works on SP and Activation) so it goes out in
            # parallel with SP's first dispatch.
            s_w22 = [None] * 6

            def w22_tile(ch, eng):
                cw = W22_G[ch] * 512
                off = sum(W22_G[:ch]) * 512
                t_ = cp.tile([128, cw], bf, name=f"s_w22_{ch}", tag=f"s_w22_{ch}")
                eng.dma_start(out=t_[:, :], in_=d_w2_2p.ap()[:, off:off + cw])
                return t_

            s_w22[1] = w22_tile(1, nc.scalar)

            # pin the table set that holds BOTH Exp and Ln so the tail Ln
            # does not pay a ~2.7us table switch
            nc.scalar.add_instruction(mybir.InstLoadActFuncSet(
                name=nc.get_next_instruction_name(), ins=[], outs=[],
                act_func_set_id=6))

            # SP dispatch order == consumption order.
            s_small = cp.tile([128, SMALL_C], bf, name="s_small", tag="s_small")
            nc.sync.dma_start(out=s_small[:, 0:1184], in_=d_small.ap()[:, 0:1184])
            nc.sync.dma_start(out=s_small[:, 1184:2720],
                              in_=d_small.ap()[:, 1184:2720])
            s_w22[2] = w22_tile(2, nc.sync)
            s_hwT = cp.tile([128, 4096], bf, name="s_hwT", tag="s_hwT")
            nc.sync.dma_start(out=s_hwT[:, :], in_=d_hwT.ap())
            nc.sync.dma_start(out=s_small[:, 2720:SMALL_C],
                              in_=d_small.ap()[:, 2720:SMALL_C])
            s_w22[3] = w22_tile(3, nc.sync)
            s_w22[4] = w22_tile(4, nc.sync)
            s_w22[5] = w22_tile(5, nc.sync)
            s_w20 = cp.tile([128, P0], bf, name="s_w20", tag="s_w20")
            nc.sync.dma_start(out=s_w20[:, :], in_=d_w2_0T.ap())
            s_w22[0] = w22_tile(0, nc.sync)
            s_w21 = cp.tile([128, Q1], bf, name="s_w21", tag="s_w21")
            nc.sync.dma_start(out=s_w21[:, :], in_=d_w2_1p.ap())
            s_mc = cp.tile([128, 6], f32, name="s_mc", tag="s_mc")
            nc.sync.dma_start(out=s_mc[:, :], in_=d_mc.ap())

            s_w1p = s_small[:, 0:672]
            s_xT = s_small[:, 2720:3744]
            s_grT = s_small[:, 3744:4768]
            s_g2T = s_small[:, 4768:5280]
            s_ones = s_small[:, 5280:5281]

            # ------------- views ----------------------------------------------
            def xm(m, k):        # masked-x chunk [128, 128]: m in A2,A0,B1,B0
                base = {0: 672, 1: 1696, 2: 1184, 3: 2208}[m]
                return s_small[:, base + k * 128: base + k * 128 + 128]

            def xTk(k, sl):      # xT chunk [128, 128] for slot range sl
                return s_xT[:, k * 256 + sl * 128: k * 256 + sl * 128 + 128]

            def hwk(k, f):       # head weight chunk [128, 512]
                o = (f * 4 + k) * 512
                return s_hwT[:, o: o + 512]

            def w1k(seg, k):     # w1 seg (0:128 | 1:32 | 2:8) chunk
                off = [0, 128, 160][seg]
                w = HSZ[seg]
                return s_w1p[:, k * 168 + off: k * 168 + off + w]

            mA2_v, mA0_v = s_mc[:, 0:1], s_mc[:, 1:2]
            mB1_v, mB0_v = s_mc[:, 2:3], s_mc[:, 3:4]
            corrA_v, corrB_v = s_mc[:, 4:5], s_mc[:, 5:6]

            # accumulators / combine tiles
            accA = ap_.tile([128, ACC_COLS], f32, name="accA", tag="accA")
            accB = ap_.tile([128, ACC_COLS], f32, name="accB", tag="accB")
            accH = ap_.tile([128, 4], f32, name="accH", tag="accH")
            tgt4 = ap_.tile([128, 4], f32, name="tgt4", tag="tgt4")
            S4 = ap_.tile([128, 4], f32, name="S4", tag="S4")
            ln4 = ap_.tile([128, 4], f32, name="ln4", tag="ln4")
            out4 = ap_.tile([128, 4], f32, name="out4", tag="out4")
            tmp = [ap_.tile([128, 1], f32, name=f"tmp{i}", tag=f"tmp{i}")
                   for i in range(6)]
            junk = ap_.tile([128, ACC_COLS], f32, name="junk", tag="junk")
            s_h2a = ap_.tile([128, 128], bf, name="s_h2a", tag="s_h2a")
            s_h1b = ap_.tile([128, 128], bf, name="s_h1b", tag="s_h1b")
            s_h0a = ap_.tile([128, 128], bf, name="s_h0a", tag="s_h0a")
            s_h0b = ap_.tile([128, 128], bf, name="s_h0b", tag="s_h0b")

            w22_bounds = np.cumsum([0] + W22_G).tolist()

            with tc.tile_pool(name="psR", bufs=2, space="PSUM") as pr:
                # ---- early projections live in one rotating ring slot so
                # they overlap the first ring groups without extra PSUM.
                # Phase 1: only h2a (gates ring A); phase 2 after ring start.
                pe_t = pr.tile([128, GRP], f32, name="proj_ps", tag="ring")
                h2_ps = pe_t[:, 0:128]
                h1_ps = pe_t[:, 128:256]
                h0a_ps = pe_t[:, 256:384]
                h0b_ps = pe_t[:, 384:512]
                for g in range(4):
                    for k in range(4):
                        nc.tensor.matmul(
                            h2_ps[32 * g:32 * g + 8, :], w1k(2, k), xm(0, k),
                            start=(k == 0), stop=(k == 3),
                            tile_position=(0, 32 * g))
                for g in range(4):
                    nc.vector.tensor_copy(s_h2a[32 * g:32 * g + 8, :],
                                          h2_ps[32 * g:32 * g + 8, :])

                # ------------- tail rings: exp + row-sum over cluster logits ---
                # ring A: cluster-2 quarters + cluster-0 overflow (groups < G0).
                # Groups >= G0 (no w2_0 dependency) run first so the ring can
                # start as soon as the first w2_2 chunk lands.

                def exp_and_sum(pt, fw, acc_col):
                    # ScalarE exp with fused row-sum accumulate (the DVE's
                    # reduce variant only runs at 1x, so ACT keeps the sum).
                    rsc = sp.tile([128, GRP], bf, name="rscr", tag="rscr")
                    if fw == 512:
                        src, dst = pt[:, :], rsc[:, :]
                    else:
                        src = pt.rearrange("p (b e) -> p b e", b=4)[:, :, 0:fw]
                        dst = rsc.rearrange("p (b e) -> p b e", b=4)[:, :, 0:fw]
                    nc.scalar.activation(dst, src, AF.Exp, accum_out=acc_col)

                def ringA_group(t):
                    ch = next(i for i in range(len(W22_G))
                              if w22_bounds[i + 1] > t)
                    w = t - w22_bounds[ch]
                    # last group: only 12500 % 512 = 212 real cols per quarter
                    fw = 212 if t == G2 - 1 else 512
                    pt = pr.tile([128, GRP], f32, name="ringA", tag="ring")
                    for g in range(4):
                        nc.tensor.matmul(
                            pt[:, g * 512:g * 512 + fw],
                            s_h2a[32 * g:32 * g + 8, :],
                            s_w22[ch][32 * g:32 * g + 8, w * 512:w * 512 + fw],
                            start=True, stop=(t >= G0),
                            tile_position=(32 * g, 0))
                    if t < G0:
                        for g in range(4):
                            nc.tensor.matmul(
                                pt[:, g * 512:(g + 1) * 512],
                                s_h0a[:, :],
                                s_w20[:, t * GRP + g * 512:t * GRP + (g + 1) * 512],
                                start=False, stop=True)
                    exp_and_sum(pt, fw, accA[:, t:t + 1])

                def head_tile(s):
                    # head logits + exp-sum for sample tile s; one ring slot,
                    # PE cost hides under ring ACT slack
                    hp = pr.tile([128, PH], f32, name=f"head_ps{s}", tag="ring")
                    for f in range(PH // 512):
                        for k in range(4):
                            nc.tensor.matmul(
                                hp[:, f * 512:(f + 1) * 512],
                                xTk(k, s), hwk(k, f),
                                start=(k == 0), stop=(k == 3))
                    hsc = sp.tile([128, PH], bf, name="hscr", tag="rscr")
                    nc.scalar.activation(hsc[:, :], hp[:, :], AF.Exp,
                                         accum_out=accH[:, s:s + 1])

                # ring A starts on the first w22 chunk; remaining projections
                # (phase 2) run on the PE under the first groups' exp time.
                ringA_group(5)
                for g in range(4):
                    for k in range(4):
                        nc.tensor.matmul(
                            h1_ps[32 * g:32 * g + 32, :], w1k(1, k), xm(2, k),
                            start=(k == 0), stop=(k == 3),
                            tile_position=(0, 32 * g))
                for k in range(4):
                    nc.tensor.matmul(h0a_ps[:, :], w1k(0, k), xm(1, k),
                                     start=(k == 0), stop=(k == 3))
                for k in range(4):
                    nc.tensor.matmul(h0b_ps[:, :], w1k(0, k), xm(3, k),
                                     start=(k == 0), stop=(k == 3))
                for g in range(4):
                    nc.vector.tensor_copy(s_h1b[32 * g:32 * g + 32, :],
                                          h1_ps[32 * g:32 * g + 32, :])
                nc.vector.tensor_copy(s_h0a[:, :], h0a_ps[:, :])
                nc.vector.tensor_copy(s_h0b[:, :], h0b_ps[:, :])

                for t in range(6, G2):
                    ringA_group(t)
                    if t == 9:
                        head_tile(0)
                    elif t == 11:
                        head_tile(1)

                # p-products for the target dots (DVE is idle here; the dot
                # matmuls themselves run after ring B)
                ph = sp.tile([128, 1024], bf, name="ph", tag="ph")
                nc.vector.tensor_mul(ph[:, :], s_xT[:, :], s_grT[:, :])
                p2 = sp.tile([128, 128], bf, name="p2", tag="p2")
                nc.vector.tensor_mul(p2[0:8, :], s_h2a[0:8, :], s_g2T[0:8, 0:128])
                p0a = sp.tile([128, 128], bf, name="p0a", tag="p0a")
                nc.vector.tensor_mul(p0a[:, :], s_h0a[:, :], s_g2T[:, 128:256])
                p1 = sp.tile([128, 128], bf, name="p1", tag="p1")
                nc.vector.tensor_mul(p1[0:32, :], s_h1b[0:32, :], s_g2T[0:32, 256:384])
                p0b = sp.tile([128, 128], bf, name="p0b", tag="p0b")
                nc.vector.tensor_mul(p0b[:, :], s_h0b[:, :], s_g2T[:, 384:512])

                def ringB_group(t):
                    # last group: only 10000 % 512 = 272 real cols per quarter
                    fw = 272 if t == G1 - 1 else 512
                    pt = pr.tile([128, GRP], f32, name="ringB", tag="ring")
                    for g in range(4):
                        nc.tensor.matmul(
                            pt[:, g * 512:g * 512 + fw],
                            s_h1b[32 * g:32 * g + 32, :],
                            s_w21[32 * g:32 * g + 32, t * 512:t * 512 + fw],
                            start=True, stop=(t >= G0),
                            tile_position=(32 * g, 0))
                    if t < G0:
                        for g in range(4):
                            nc.tensor.matmul(
                                pt[:, g * 512:(g + 1) * 512],
                                s_h0b[:, :],
                                s_w20[:, t * GRP + g * 512:t * GRP + (g + 1) * 512],
                                start=False, stop=True)
                    exp_and_sum(pt, fw, accB[:, t:t + 1])

                for t in range(G0):
                    ringA_group(t)

                # ring-A side of the combine (DVE; runs while ring B exps):
                # S_A = mA2*sum(accA[:, :G2]) + mA0*sum(accA[:, :G0]) + corrA,
                # fused as two masked tensor_scalar reduces + one 2-op add.
                nc.vector.tensor_scalar(
                    out=junk[:, 0:G2], in0=accA[:, 0:G2], scalar1=mA2_v,
                    scalar2=None, op0=ALU.mult, op1=ALU.add, accum_out=tmp[0][:, :])
                nc.vector.tensor_scalar(
                    out=junk[:, 0:G0], in0=accA[:, 0:G0], scalar1=mA0_v,
                    scalar2=None, op0=ALU.mult, op1=ALU.add, accum_out=tmp[1][:, :])
                nc.vector.tensor_scalar(
                    out=S4[:, 2:3], in0=tmp[0][:, :], scalar1=tmp[1][:, :],
                    scalar2=corrA_v, op0=ALU.add, op1=ALU.add)
                # head: S = accH - (PH - 1003) pad-exp correction
                nc.vector.tensor_scalar(
                    out=S4[:, 0:2], in0=accH[:, 0:2], scalar1=float(-(PH - 1003)),
                    scalar2=None, op0=ALU.add)

                for t in range(7):
                    ringB_group(t)

                # target-logit dots: per-slot dot(u, v) = (u .* v)^T @ ones
                # (partition-dim contraction on the PE -> [slots, 1] PSUM);
                # one slot in ring B's light region, hidden under its ACT slack
                dots_ps = pr.tile([128, GRP], f32, name="dots_ps", tag="ring")
                for k in range(4):
                    nc.tensor.matmul(dots_ps[:, 0:1], ph[:, k * 256:k * 256 + 128],
                                     s_ones[:, :], start=(k == 0), stop=(k == 3))
                for k in range(4):
                    nc.tensor.matmul(dots_ps[:, 1:2],
                                     ph[:, k * 256 + 128:k * 256 + 256],
                                     s_ones[:, :], start=(k == 0), stop=(k == 3))
                nc.tensor.matmul(dots_ps[:, 2:3], p2[0:8, :], s_ones[0:8, :],
                                 start=True, stop=False)
                nc.tensor.matmul(dots_ps[:, 2:3], p0a[:, :], s_ones[:, :],
                                 start=False, stop=True)
                nc.tensor.matmul(dots_ps[:, 3:4], p1[0:32, :], s_ones[0:32, :],
                                 start=True, stop=False)
                nc.tensor.matmul(dots_ps[:, 3:4], p0b[:, :], s_ones[:, :],
                                 start=False, stop=True)
                nc.vector.tensor_copy(tgt4[:, :], dots_ps[:, 0:4])

                for t in range(7, G1):
                    ringB_group(t)

            # ------------- combine (only ring-B accB remains) ----------------
            nc.vector.tensor_scalar(
                out=junk[:, 0:G1], in0=accB[:, 0:G1], scalar1=mB1_v,
                scalar2=None, op0=ALU.mult, op1=ALU.add, accum_out=tmp[2][:, :])
            nc.vector.tensor_scalar(
                out=junk[:, 0:G0], in0=accB[:, 0:G0], scalar1=mB0_v,
                scalar2=None, op0=ALU.mult, op1=ALU.add, accum_out=tmp[3][:, :])
            nc.vector.tensor_scalar(
                out=S4[:, 3:4], in0=tmp[2][:, :], scalar1=tmp[3][:, :],
                scalar2=corrB_v, op0=ALU.add, op1=ALU.add)
            nc.scalar.activation(ln4[:, :], S4[:, :], AF.Ln)
            nc.vector.tensor_sub(out4[:, :], tgt4[:, :], ln4[:, :])
            nc.sync.dma_start(out=d_out.ap(), in_=out4[:, :])

    nc.compile()
    return nc


def _get_nc():
    global _BUILT
    if _BUILT is None:
        _BUILT = build_nc()
    return _BUILT


# ================================ entry point ================================

def _numpy_fallback(inputs):
    """Last-resort exact computation (only if the slot assignment misfits,
    which cannot happen for the deterministic problem inputs)."""
    x = np.asarray(inputs["user_repr"], np.float64)
    t = np.asarray(inputs["targets"]).astype(np.int64)
    head_w = np.asarray(inputs["head_w"], np.float64)
    rows = np.arange(x.shape[0])

    def lse_rows(logits):
        m = logits.max(axis=1, keepdims=True)
        return (np.log(np.exp(logits - m).sum(axis=1, keepdims=True)) + m)

    hl = x @ head_w.T
    head_lp = hl - lse_rows(hl)
    out = np.where(t < SHORT, head_lp[rows, np.minimum(t, SHORT - 1)], 0.0)
    for i in range(3):
        w1 = np.asarray(inputs[f"tail_w1_{i}"], np.float64)
        w2 = np.asarray(inputs[f"tail_w2_{i}"], np.float64)
        tl = (x @ w1.T) @ w2.T
        tail_lp = tl - lse_rows(tl)
        rel = np.clip(t - CUT[i], 0, CUT[i + 1] - CUT[i] - 1)
        val = head_lp[:, SHORT + i] + tail_lp[rows, rel]
        out = np.where((t >= CUT[i]) & (t < CUT[i + 1]), val, out)
    return out.astype(np.float32)


def kernel(**inputs):
    from concourse.bass_utils import run_bass_kernel_spmd

    targets = np.asarray(inputs["targets"]).astype(np.int64)
    try:
        tileA, tileB, cl = _assign(targets)
    except AssertionError:
        return _numpy_fallback(inputs)
    in_maps = _host_arrays(inputs, tileA, tileB, cl)
    nc = _get_nc()
    res = run_bass_kernel_spmd(nc, in_maps, core_ids=list(range(NCORES)))
    out = np.zeros(N, np.float32)
    for c in range(NCORES):
        o = res.results[c]["out"]   # [128, 4]
        for s, i in enumerate(tileA[c]):
            if i >= 0:
                out[i] = o[s, 0] + (o[s, 2] if cl[i] >= 0 else 0.0)
        for s, i in enumerate(tileB[c]):
            if i >= 0:
                out[i] = o[s, 1] + o[s, 3]
    return out
